# revision 1
# baseline (speedup 1.0000x reference)
"""DeepReservoir (leaky ESN, 4 modules) Trainium2 Bass kernel.

Problem: h[t] = (1-a)*h[t-1] + a*tanh(u[t] @ Kin + h[t-1] @ W + bias) per
module, T=8192 steps, U=1024 units, a=0.9, batch 1.  Output = all states,
modules concatenated on the feature axis: [1, T, 4*1024].

Strategy (module parallel, per the sharding hint):
  - One reservoir module per NeuronCore (4 modules; cores 4-7 run duplicates
    so one SPMD program serves all 8 cores; host gathers from cores 0-3).
  - The input projection c[t] = u[t] @ Kin + bias has no time dependence and
    is tiny (4 GFLOP total): computed on the host, shipped pre-swizzled into
    the exact per-chunk SBUF layout so the per-iteration DMA is one fully
    contiguous 128KB block (dynamic strided layouts cost ~80us/DMA in
    descriptor processing - measured).
  - The time scan is the serial bottleneck: per step a [1024]x[1024,1024]
    matvec on TensorE as 64 LDWEIGHTS+MATMUL pairs of [128,128]x[128,1]
    (weight-load bound, ~70-100ns/pair).  Weights are bf16 (enables
    fast-weight-load) with leaky a folded in: W' = a*W.  State is kept fp32
    via the rescaled recurrence h'[t] = (1-a)*h'[t-1] + tanh(W' h'[t-1] +
    c[t]); the output is a*h'.
  - Per step the matmuls are phase-ordered (contraction tiles 0-3 for all
    output tiles, then finish output tiles 0-3, then 4-7) so ScalarE/VectorE
    process the first half of the new state while TensorE finishes the
    second half, and the next step's matmuls (which need only the first
    half as contraction input) start immediately -> TensorE stays busy.
  - tanh on ScalarE (one [128,4] op per half), z+c add and leaky blends on
    VectorE; the bf16 copy of the new state is written first to unblock
    TensorE.
  - Output states are staged in SBUF and DMAd per 32-step chunk in the
    SBUF-native layout; the host inverts the layout after gathering.
"""

import numpy as np
import ml_dtypes

import concourse.bacc as bacc
import concourse.tile as tile
import concourse.mybir as mybir
from concourse.bass import ds
from concourse.bass_utils import run_bass_kernel_spmd

F32 = mybir.dt.float32
BF16 = mybir.dt.bfloat16

UNITS = 1024
IN = 64
KT = 8  # contraction tiles (1024/128)
MT = 8  # output-unit tiles (1024/128)
P = 128

LEAKY = np.float32(0.9)
ONE_MINUS_LEAKY = float(np.float32(1.0) - np.float32(0.9))

N_CORES = 8
N_MODULES = 4


def build_nc(T: int, unroll: int):
    """Build the single-core SPMD Bass program for one reservoir module."""
    assert T % unroll == 0 and unroll % 2 == 0
    nchunk = T // unroll
    nc = bacc.Bacc("TRN2", debug=False)

    wT = nc.dram_tensor("wT", [UNITS, UNITS], BF16, kind="ExternalInput")
    # c pre-swizzled on host: c_in[chunk, p, s, j] = c[chunk*unroll+s, j*128+p]
    c_in = nc.dram_tensor("c_in", [nchunk, P, unroll, MT], F32, kind="ExternalInput")
    # output in SBUF-native layout: hs[chunk, p, s, j] = h[chunk*unroll+s, j*128+p]
    hs = nc.dram_tensor("hs", [nchunk, P, unroll, MT], F32, kind="ExternalOutput")

    with tile.TileContext(nc) as tc:
        with (
            tc.tile_pool(name="const", bufs=1) as const_pool,
            tc.tile_pool(name="cin", bufs=2) as cin_pool,
            tc.tile_pool(name="hout", bufs=2) as hout_pool,
            tc.tile_pool(name="work", bufs=2) as work_pool,
            tc.tile_pool(name="zpsum", bufs=2, space="PSUM") as zpsum_pool,
        ):
            # weights: w_sb[p, k, m, c] = W'[k*128+p, m*128+c]
            w_sb = const_pool.tile([P, KT, MT, P], BF16)
            nc.sync.dma_start(
                w_sb[:], wT[:, :].rearrange("(k p) (m c) -> p k m c", p=P, c=P)
            )

            # persistent scan state (ping-pong on dim 1 by step parity)
            hstate = const_pool.tile([P, 2, MT], F32)  # h' fp32 master
            h16 = const_pool.tile([P, 2, MT], BF16)  # bf16 copy for PE rhs
            nc.vector.memset(hstate[:, 1, :], 0.0)
            nc.vector.memset(h16[:, 1, :], 0.0)

            c_v = c_in[:, :, :, :].rearrange("c p s j -> p c s j")
            hs_v = hs[:, :, :, :].rearrange("c p s j -> p c s j")

            with tc.For_i(
                0,
                nchunk,
                1,
                hint_engines=(mybir.EngineType.PE, mybir.EngineType.Activation),
            ) as iv:
                cchunk = cin_pool.tile([P, unroll, MT], F32, tag="cchunk")
                nc.sync.dma_start(cchunk[:], c_v[:, ds(iv, 1), :, :])
                hstage = hout_pool.tile([P, unroll, MT], F32, tag="hstage")

                for s in range(unroll):
                    cur = s % 2
                    prev = 1 - cur
                    zA = zpsum_pool.tile([P, 4], F32, tag="zA")
                    zB = zpsum_pool.tile([P, 4], F32, tag="zB")

                    def mm(k, m, start, stop):
                        zt = zA if m < 4 else zB
                        nc.tensor.matmul(
                            zt[:, (m % 4) : (m % 4) + 1],
                            w_sb[:, k, m, :],
                            h16[:, prev, k : k + 1],
                            start=start,
                            stop=stop,
                        )

                    # phase 1: contraction tiles 0-3 (only needs half A of
                    # h16, which the previous step produced early)
                    for k in range(4):
                        for m in range(MT):
                            mm(k, m, start=(k == 0 and m % 4 == 0), stop=False)
                    # phase 2a: finish z columns 0-3 so ScalarE can start
                    for m in range(4):
                        for k in range(4, 8):
                            mm(k, m, start=False, stop=(k == 7 and m == 3))
                    # phase 2b: finish z columns 4-7
                    for m in range(4, 8):
                        for k in range(4, 8):
                            mm(k, m, start=False, stop=(k == 7 and m == 7))

                    zc = work_pool.tile([P, MT], F32, tag="zc")
                    o32 = work_pool.tile([P, MT], F32, tag="o32")
                    for (lo, hi), zt in (((0, 4), zA), ((4, 8), zB)):
                        # zc = z + c[t]
                        nc.vector.tensor_add(
                            zc[:, lo:hi], zt[:, 0:4], cchunk[:, s, lo:hi]
                        )
                        # o = tanh(zc)
                        nc.scalar.activation(
                            o32[:, lo:hi],
                            zc[:, lo:hi],
                            mybir.ActivationFunctionType.Tanh,
                        )
                        # critical-path first: bf16 state for the next matmuls
                        nc.vector.scalar_tensor_tensor(
                            out=h16[:, cur, lo:hi],
                            in0=hstate[:, prev, lo:hi],
                            scalar=ONE_MINUS_LEAKY,
                            in1=o32[:, lo:hi],
                            op0=mybir.AluOpType.mult,
                            op1=mybir.AluOpType.add,
                        )
                        # fp32 master state (off critical path)
                        nc.vector.scalar_tensor_tensor(
                            out=hstate[:, cur, lo:hi],
                            in0=hstate[:, prev, lo:hi],
                            scalar=ONE_MINUS_LEAKY,
                            in1=o32[:, lo:hi],
                            op0=mybir.AluOpType.mult,
                            op1=mybir.AluOpType.add,
                        )
                    # output h[t] = a * h'[t]
                    nc.vector.tensor_scalar_mul(
                        hstage[:, s, :], hstate[:, cur, :], float(LEAKY)
                    )

                nc.sync.dma_start(hs_v[:, ds(iv, 1), :, :], hstage[:])

    nc.compile()
    return nc


def _prep_in_maps(u, kernel, rec_kernel, bias, T, unroll):
    nchunk = T // unroll
    u0 = np.asarray(u[0], dtype=np.float32)  # [T, 64]
    in_maps = []
    for core in range(N_CORES):
        m = core % N_MODULES
        wT = np.ascontiguousarray(
            (np.asarray(rec_kernel[m], dtype=np.float32) * LEAKY).astype(
                ml_dtypes.bfloat16
            )
        )
        # c[t, u] = u[t] @ Kin + bias  (fp32, host)
        c = u0 @ np.asarray(kernel[m], dtype=np.float32) + np.asarray(
            bias[m], dtype=np.float32
        )
        # -> c_in[chunk, p, s, j]
        c_sw = np.ascontiguousarray(
            c.reshape(nchunk, unroll, MT, P).transpose(0, 3, 1, 2)
        )
        in_maps.append({"wT": wT, "c_in": c_sw})
    return in_maps


def _unswizzle(hs_dev, T, unroll):
    # hs_dev[chunk, p, s, j] -> [T, 1024] with unit u = j*128+p
    nchunk = T // unroll
    return np.ascontiguousarray(
        hs_dev.transpose(0, 2, 3, 1).reshape(T, UNITS)
    )


_NC_CACHE = {}


def run(u, kernel, rec_kernel, bias, unroll=32, trace=False):
    T = u.shape[1]
    key = (T, unroll)
    if key not in _NC_CACHE:
        _NC_CACHE[key] = build_nc(T, unroll)
    nc = _NC_CACHE[key]
    in_maps = _prep_in_maps(u, kernel, rec_kernel, bias, T, unroll)
    res = run_bass_kernel_spmd(
        nc, in_maps, core_ids=list(range(N_CORES)), trace=trace
    )
    out = np.concatenate(
        [_unswizzle(res.results[m]["hs"], T, unroll) for m in range(N_MODULES)],
        axis=1,
    )  # [T, 4096]
    return out[None].astype(np.float32), res


def kernel(u, kernel, rec_kernel, bias):
    out, _ = run(u, kernel, rec_kernel, bias)
    return out

